# revision 1
# baseline (speedup 1.0000x reference)
"""MultiHeadAttention Trainium2 kernel (8 NeuronCores, Bass/Tile).

Problem: B=2, S=2048, D=1024, H=16, DK=64 fp32 MHA (torch-Linear style
projections, softmax attention, output projection).

Sharding: core c = (batch b = c//4, head-group g = c%4); each core handles
4 heads of one batch, entirely in a transposed layout (features on
partitions, sequence on the free axis):
  qhT/khT  = (W_g x^T + b)       [2 pairs x 128, 2048]
  vh       = x_v Wv_g^T          [2048, 4x65] (ones col -> row sums)
  scoresT  = khT^T qhT           per (pair, ktile, qtile) -> PSUM
  expT     = exp(scoresT/8 - 2)  ACT (bias -2 for fp16 headroom)
  rawT     = vh_aug^T expT       PV matmul; row 64 = softmax denominator
  outT     = rawT[0:64] * (1/rawT[64])
  partialT = woT^T outT          [1024, 2048] fp32 -> DRAM
Host: out[b] = sum_g partialT(b,g)^T + (Wo bv + bo).

PE is exact on fp16/bf16 operands (fp32 accumulate); per-stage operand
dtypes are configurable below. Softmax denominators come free via the
ones column (attention rows sum to 1, which also lets Wo@bv fold into a
host-side constant). No collectives; host sums 4 partials per batch.
"""

import numpy as np

B, S, D, H = 2, 2048, 1024, 16
DK = D // H          # 64
N_CORES = 8
HG = H // 4          # 4 head-groups
HL = 4               # heads per core
FEAT = HL * DK       # 256 per-core features
NQT = S // 512       # 4 query tiles
NKT = S // 128       # 16 key tiles
NDT = D // 128       # 8 contraction tiles (d-model)

# per-stage matmul operand dtypes ("fp16" | "bf16")
DT_QK = "fp16"   # x_q/x_k, Wq/Wk, qhT/khT (score operands)
DT_V = "fp16"    # x_v, Wv
DT_PV = "bf16"   # vh_aug, expT
DT_O = "fp16"    # Wo, outT (feeds final output directly)

_cache = {}


def _np_dt(name):
    if name == "fp16":
        return np.float16
    import ml_dtypes
    return ml_dtypes.bfloat16


def _build():
    import concourse.mybir as mybir
    import concourse.tile as tile
    from concourse import bacc

    fp32 = mybir.dt.float32
    dt_qk = getattr(mybir.dt, "float16" if DT_QK == "fp16" else "bfloat16")
    dt_v = getattr(mybir.dt, "float16" if DT_V == "fp16" else "bfloat16")
    dt_pv = getattr(mybir.dt, "float16" if DT_PV == "fp16" else "bfloat16")
    dt_o = getattr(mybir.dt, "float16" if DT_O == "fp16" else "bfloat16")

    nc = bacc.Bacc("TRN2", target_bir_lowering=False, debug=False,
                   num_devices=N_CORES)

    xqT = nc.dram_tensor("xqT", [D, S], dt_qk, kind="ExternalInput").ap()
    xkT = nc.dram_tensor("xkT", [D, S], dt_qk, kind="ExternalInput").ap()
    xvT = nc.dram_tensor("xvT", [D, S], dt_v, kind="ExternalInput").ap()
    wqT = nc.dram_tensor("wqT", [D, FEAT], dt_qk, kind="ExternalInput").ap()
    wkT = nc.dram_tensor("wkT", [D, FEAT], dt_qk, kind="ExternalInput").ap()
    wvT = nc.dram_tensor("wvT", [D, FEAT], dt_v, kind="ExternalInput").ap()
    woT = nc.dram_tensor("woT", [FEAT, D], dt_o, kind="ExternalInput").ap()
    bq2 = nc.dram_tensor("bq2", [FEAT, 1], fp32, kind="ExternalInput").ap()
    bk2 = nc.dram_tensor("bk2", [FEAT, 1], fp32, kind="ExternalInput").ap()
    out_d = nc.dram_tensor("partialT", [D, S], fp32, kind="ExternalOutput").ap()

    xq_r = xqT.rearrange("(t p) s -> p t s", p=128)
    xk_r = xkT.rearrange("(t p) s -> p t s", p=128)
    xv_r = xvT.rearrange("(t p) s -> p t s", p=128)

    with tile.TileContext(nc) as tc:
        with (
            tc.tile_pool(name="xin", bufs=1) as xin,
            tc.tile_pool(name="win", bufs=1) as win,
            tc.tile_pool(name="proj", bufs=1) as proj,
            tc.tile_pool(name="pexp", bufs=17) as pexp,
            tc.tile_pool(name="pout", bufs=4) as pout,
            tc.tile_pool(name="pnrm", bufs=2) as pnrm,
            tc.tile_pool(name="pp", bufs=2, space="PSUM") as pp,
            tc.tile_pool(name="ps2", bufs=2, space="PSUM") as ps2,
            tc.tile_pool(name="pspv", bufs=2, space="PSUM") as pspv,
        ):
            # ---- load inputs: weights first (small, unblock compute),
            # then x per d-tile in consumption order (v, then q/k) ----
            wq3 = win.tile([128, NDT, FEAT], dt_qk, tag="wq")
            wk3 = win.tile([128, NDT, FEAT], dt_qk, tag="wk")
            wv3 = win.tile([128, NDT, FEAT], dt_v, tag="wv")
            wo3 = win.tile([128, 2, D], dt_o, tag="wo")
            bq3 = win.tile([128, 2, 1], fp32, tag="bq")
            bk3 = win.tile([128, 2, 1], fp32, tag="bk")
            nc.sync.dma_start(wk3[:], wkT.rearrange("(t p) f -> p t f", p=128))
            nc.sync.dma_start(wq3[:], wqT.rearrange("(t p) f -> p t f", p=128))
            nc.sync.dma_start(wv3[:], wvT.rearrange("(t p) f -> p t f", p=128))
            nc.sync.dma_start(wo3[:], woT.rearrange("(t p) j -> p t j", p=128))
            nc.sync.dma_start(bq3[:], bq2.rearrange("(t p) o -> p t o", p=128))
            nc.sync.dma_start(bk3[:], bk2.rearrange("(t p) o -> p t o", p=128))
            xq3 = xin.tile([128, NDT, S], dt_qk, tag="xq")
            xk3 = xin.tile([128, NDT, S], dt_qk, tag="xk")
            xv3 = xin.tile([128, NDT, S], dt_v, tag="xv")
            for t in range(NDT):
                nc.sync.dma_start(xk3[:, t, :], xk_r[:, t, :])
                nc.sync.dma_start(xq3[:, t, :], xq_r[:, t, :])
            for t in range(NDT):
                nc.sync.dma_start(xv3[:, t, :], xv_r[:, t, :])

            # ---- persistent intermediates ----
            qh3 = proj.tile([128, 2, S], dt_qk, tag="qh")   # pair-packed
            kh3 = proj.tile([128, 2, S], dt_qk, tag="kh")
            vha = proj.tile([128, NKT, HL, DK + 1], dt_pv, tag="vha")
            ot3 = proj.tile([128, 2, S], dt_o, tag="outT")

            nc.gpsimd.memset(vha[:, :, :, DK], 1.0)  # ones column
            # exp bias -2: headroom under fp16 max (cancels in division)
            ebias = win.tile([128, 1], fp32, tag="ebias")
            nc.gpsimd.memset(ebias[:], -2.0)

            # ---- projections ----
            # emission order: q/k for pair 0 first, then v, then q/k pair 1 —
            # pair-0 scores/exp become schedulable early, keeping ACT busy
            # while the remaining projections still occupy the PE.
            def qk_proj(m):
                for x3, w3, b3, dst in ((xk3, wk3, bk3, kh3),
                                        (xq3, wq3, bq3, qh3)):
                    for n in range(NQT):
                        ps = pp.tile([128, 512], fp32, tag="acc")
                        for kt in range(NDT):
                            nc.tensor.matmul(
                                ps[:],
                                w3[:, kt, m * 128:(m + 1) * 128],
                                x3[:, kt, n * 512:(n + 1) * 512],
                                start=(kt == 0), stop=(kt == NDT - 1))
                        nc.vector.tensor_scalar_add(
                            dst[:, m, n * 512:(n + 1) * 512], ps[:], b3[:, m, :])

            def v_proj():
                for st in range(NKT):
                    ps = pp.tile([128, 256], fp32, tag="acc")
                    for kt in range(NDT):
                        nc.tensor.matmul(
                            ps[:], xv3[:, kt, st * 128:(st + 1) * 128],
                            wv3[:, kt, :],
                            start=(kt == 0), stop=(kt == NDT - 1))
                    nc.vector.tensor_copy(vha[:, st, :, 0:DK], ps[:])

            # ---- attention (split so scores/exp of (0,0) can be
            # emitted before v-proj and qk_proj(1), starting ACT ~35us
            # earlier; PV readers are emitted only after v-proj writes) ----
            def attn_scores(qt, hp):
                e2s = []
                for kt in range(NKT):
                    s2 = ps2.tile([128, 1024], fp32, tag="s2")
                    nc.tensor.matmul(
                        s2[:, 0:512],
                        kh3[0:64, hp, kt * 128:(kt + 1) * 128],
                        qh3[0:64, hp, qt * 512:(qt + 1) * 512],
                        start=True, stop=True)
                    nc.tensor.matmul(
                        s2[:, 512:1024],
                        kh3[64:128, hp, kt * 128:(kt + 1) * 128],
                        qh3[64:128, hp, qt * 512:(qt + 1) * 512],
                        start=True, stop=True)
                    e2 = pexp.tile([128, 1024], dt_pv, tag="e2")
                    if DT_PV == "bf16":   # bf16 range: no overflow risk
                        nc.scalar.activation(
                            e2[:], s2[:],
                            mybir.ActivationFunctionType.Exp, scale=0.125)
                    else:
                        nc.scalar.activation(
                            e2[:], s2[:],
                            mybir.ActivationFunctionType.Exp,
                            scale=0.125, bias=ebias[:])
                    e2s.append(e2)
                return e2s

            def attn_pv(qt, hp, e2s):
                pva = pspv.tile([DK + 1, 512], fp32, tag="pv")
                pvb = pspv.tile([DK + 1, 512], fp32, tag="pv")
                for kt in range(NKT):
                    nc.tensor.matmul(
                        pva[:], vha[:, kt, 2 * hp, :], e2s[kt][:, 0:512],
                        start=(kt == 0), stop=(kt == NKT - 1))
                    nc.tensor.matmul(
                        pvb[:], vha[:, kt, 2 * hp + 1, :],
                        e2s[kt][:, 512:1024],
                        start=(kt == 0), stop=(kt == NKT - 1))
                for pv, half in ((pva, 0), (pvb, 1)):
                    # custom DVE ops must read SBUF, not PSUM
                    srow = pnrm.tile([1, 512], fp32, tag="srow")
                    nc.vector.tensor_copy(srow[:], pv[DK:DK + 1, :])
                    inv = pnrm.tile([1, 512], fp32, tag="inv")
                    nc.vector.reciprocal_approx_fast(inv[:], srow[:])
                    invb = pnrm.tile([64, 512], fp32, tag="invb")
                    nc.gpsimd.partition_broadcast(invb[:], inv[:])
                    nc.vector.tensor_tensor(
                        ot3[half * 64:(half + 1) * 64, hp,
                            qt * 512:(qt + 1) * 512],
                        pv[0:DK, :], invb[:], mybir.AluOpType.mult)

            def oproj(qt):
                for jt in range(NDT):
                    ps = pp.tile([128, 512], fp32, tag="acc")
                    for m in range(2):
                        nc.tensor.matmul(
                            ps[:], wo3[:, m, jt * 128:(jt + 1) * 128],
                            ot3[:, m, qt * 512:(qt + 1) * 512],
                            start=(m == 0), stop=(m == 1))
                    po = pout.tile([128, 512], fp32, tag="po")
                    nc.vector.tensor_copy(po[:], ps[:])
                    nc.sync.dma_start(
                        out_d[jt * 128:(jt + 1) * 128,
                              qt * 512:(qt + 1) * 512], po[:])

            qk_proj(0)
            e00 = attn_scores(0, 0)   # ACT starts here, during qk1/v-proj
            qk_proj(1)
            v_proj()
            attn_pv(0, 0, e00)
            e01 = attn_scores(0, 1)
            attn_pv(0, 1, e01)
            oproj(0)
            for qt in range(1, NQT):
                for hp in range(2):
                    e = attn_scores(qt, hp)
                    attn_pv(qt, hp, e)
                oproj(qt)

    nc.compile()
    return nc


def kernel(q, k, v, Wq, bq, Wk, bk, Wv, bv, Wo, bo, _trace=False):
    from concourse import bass_utils

    if "nc" not in _cache:
        _cache["nc"] = _build()
    nc = _cache["nc"]

    q = np.asarray(q, np.float32)
    k = np.asarray(k, np.float32)
    v = np.asarray(v, np.float32)
    Wq = np.asarray(Wq, np.float32)
    Wk = np.asarray(Wk, np.float32)
    Wv = np.asarray(Wv, np.float32)
    Wo = np.asarray(Wo, np.float32)
    bq = np.asarray(bq, np.float32)
    bk = np.asarray(bk, np.float32)
    bv = np.asarray(bv, np.float32)
    bo = np.asarray(bo, np.float32)

    d_qk, d_v, d_o = _np_dt(DT_QK), _np_dt(DT_V), _np_dt(DT_O)
    xT = {}
    for b in range(B):
        xT[("q", b)] = np.ascontiguousarray(q[b].T).astype(d_qk)
        xT[("k", b)] = np.ascontiguousarray(k[b].T).astype(d_qk)
        xT[("v", b)] = np.ascontiguousarray(v[b].T).astype(d_v)
    wT = {}
    for g in range(HG):
        sl = slice(g * FEAT, (g + 1) * FEAT)
        wT[("q", g)] = np.ascontiguousarray(Wq[sl, :].T).astype(d_qk)
        wT[("k", g)] = np.ascontiguousarray(Wk[sl, :].T).astype(d_qk)
        wT[("v", g)] = np.ascontiguousarray(Wv[sl, :].T).astype(d_v)
        wT[("o", g)] = np.ascontiguousarray(Wo[:, sl].T).astype(d_o)

    in_maps = []
    for c in range(N_CORES):
        b, g = divmod(c, HG)
        sl = slice(g * FEAT, (g + 1) * FEAT)
        in_maps.append({
            "xqT": xT[("q", b)], "xkT": xT[("k", b)], "xvT": xT[("v", b)],
            "wqT": wT[("q", g)], "wkT": wT[("k", g)], "wvT": wT[("v", g)],
            "woT": wT[("o", g)],
            "bq2": np.ascontiguousarray(bq[sl]).reshape(FEAT, 1),
            "bk2": np.ascontiguousarray(bk[sl]).reshape(FEAT, 1),
        })

    kwargs = {}
    if _trace:
        _install_profile_shim()
        kwargs = dict(trace=True, trace_cores=list(range(N_CORES)))
    res = bass_utils.run_bass_kernel_spmd(
        nc, in_maps, core_ids=list(range(N_CORES)), **kwargs)
    _cache["last_results"] = res

    final_bias = (Wo @ bv + bo).astype(np.float32)  # attn rows sum to 1
    out = np.empty((B, S, D), np.float32)
    for b in range(B):
        acc = res.results[b * HG]["partialT"].copy()
        for g in range(1, HG):
            acc += res.results[b * HG + g]["partialT"]
        out[b] = acc.T + final_bias
    return out


def _install_profile_shim():
    """Provide antenv.axon_hooks so trace=True works under axon."""
    import sys
    import types

    import antenv

    if "antenv.axon_hooks" in sys.modules:
        return
    mod = types.ModuleType("antenv.axon_hooks")
    mod._hook = None
    mod.set_axon_ntff_profile_hook = lambda h: setattr(mod, "_hook", h)
    mod.get_axon_ntff_profile_hook = lambda: mod._hook
    sys.modules["antenv.axon_hooks"] = mod
    antenv.axon_hooks = mod
    try:
        from trn_agent_boot.trn_boot import _ntff_profile_via_ctypes
        mod.set_axon_ntff_profile_hook(
            _ntff_profile_via_ctypes("/opt/axon/libaxon_pjrt.so"))
    except Exception:
        pass



# revision 5
# speedup vs baseline: 1.0248x; 1.0248x over previous
"""MultiHeadAttention Trainium2 kernel (8 NeuronCores, Bass/Tile).

Problem: B=2, S=2048, D=1024, H=16, DK=64 fp32 MHA (torch-Linear style
projections, softmax attention, output projection).

Sharding: core c = (batch b = c//4, head-group g = c%4); each core handles
4 heads of one batch, entirely in a transposed layout (features on
partitions, sequence on the free axis):
  qhT/khT  = (W_g x^T + b)       [2 pairs x 128, 2048]
  vh       = x_v Wv_g^T          [2048, 4x65] (ones col -> row sums)
  scoresT  = khT^T qhT           per (pair, ktile, qtile) -> PSUM
  expT     = exp(scoresT/8 - 2)  ACT (bias -2 for fp16 headroom)
  rawT     = vh_aug^T expT       PV matmul; row 64 = softmax denominator
  outT     = rawT[0:64] * (1/rawT[64])
  partialT = woT^T outT          [1024, 2048] fp16 -> DRAM
Host: out[b] = sum_g partialT(b,g)^T + (Wo bv + bo).

Schedule: the kernel is ACT(exp)-throughput-bound (128 x ~1us activates).
Emission order keeps ACT saturated: qk_proj(pair0) -> scores(0,0) ->
qk_proj(pair1) -> pre-emit part of scores(0,1) -> steady blocks where
pv(cur) is kt-interleaved with scores(next) and v_proj / o_proj units
ride in the PE slack. The two K=64 score matmuls per kt auto-row-tile
(base partitions 0/64) and run concurrently on the PE.
"""

import numpy as np

B, S, D, H = 2, 2048, 1024, 16
DK = D // H          # 64
N_CORES = 8
HG = H // 4          # 4 head-groups
HL = 4               # heads per core
FEAT = HL * DK       # 256 per-core features
NQT = S // 512       # 4 query tiles
NKT = S // 128       # 16 key tiles
NDT = D // 128       # 8 contraction tiles (d-model)

# per-stage matmul operand dtypes ("fp16" | "bf16")
DT_QK = "fp16"   # x_q/x_k, Wq/Wk, qhT/khT (score operands)
DT_V = "fp16"    # x_v, Wv
DT_PV = "bf16"   # vh_aug, expT
DT_O = "fp16"    # Wo, outT (feeds final output directly)

E2_BUFS = 20     # e2 tile ring (SBUF-limited; ACT backlog cap = bufs-16)
E01_PRE = 6      # scores(0,1) units emitted right after qk_proj(1)

_cache = {}


def _np_dt(name):
    if name == "fp16":
        return np.float16
    import ml_dtypes
    return ml_dtypes.bfloat16


def _build():
    import concourse.mybir as mybir
    import concourse.tile as tile
    from concourse import bacc

    fp32 = mybir.dt.float32
    dt_qk = getattr(mybir.dt, "float16" if DT_QK == "fp16" else "bfloat16")
    dt_v = getattr(mybir.dt, "float16" if DT_V == "fp16" else "bfloat16")
    dt_pv = getattr(mybir.dt, "float16" if DT_PV == "fp16" else "bfloat16")
    dt_o = getattr(mybir.dt, "float16" if DT_O == "fp16" else "bfloat16")

    nc = bacc.Bacc("TRN2", target_bir_lowering=False, debug=False,
                   num_devices=N_CORES)

    xqT = nc.dram_tensor("xqT", [D, S], dt_qk, kind="ExternalInput").ap()
    xkT = nc.dram_tensor("xkT", [D, S], dt_qk, kind="ExternalInput").ap()
    xvT = nc.dram_tensor("xvT", [D, S], dt_v, kind="ExternalInput").ap()
    wqT = nc.dram_tensor("wqT", [D, FEAT], dt_qk, kind="ExternalInput").ap()
    wkT = nc.dram_tensor("wkT", [D, FEAT], dt_qk, kind="ExternalInput").ap()
    wvT = nc.dram_tensor("wvT", [D, FEAT], dt_v, kind="ExternalInput").ap()
    woT = nc.dram_tensor("woT", [FEAT, D], dt_o, kind="ExternalInput").ap()
    bq2 = nc.dram_tensor("bq2", [FEAT, 1], fp32, kind="ExternalInput").ap()
    bk2 = nc.dram_tensor("bk2", [FEAT, 1], fp32, kind="ExternalInput").ap()
    out_d = nc.dram_tensor("partialT", [D, S], dt_o, kind="ExternalOutput").ap()

    xq_r = xqT.rearrange("(t p) s -> p t s", p=128)
    xk_r = xkT.rearrange("(t p) s -> p t s", p=128)
    xv_r = xvT.rearrange("(t p) s -> p t s", p=128)

    with tile.TileContext(nc) as tc:
        with (
            tc.tile_pool(name="xin", bufs=1) as xin,
            tc.tile_pool(name="win", bufs=1) as win,
            tc.tile_pool(name="proj", bufs=1) as proj,
            tc.tile_pool(name="pexp", bufs=E2_BUFS) as pexp,
            tc.tile_pool(name="pout", bufs=2) as pout,
            tc.tile_pool(name="pnrm", bufs=2) as pnrm,
            tc.tile_pool(name="pp", bufs=2, space="PSUM") as pp,
            tc.tile_pool(name="ps2", bufs=2, space="PSUM") as ps2,
            tc.tile_pool(name="pspv", bufs=2, space="PSUM") as pspv,
        ):
            # ---- load inputs. Order = consumption order: q/k weights,
            # then xk/xq interleaved (first exp gates on these), then the
            # v path, then wo (only needed ~60us in).
            wq3 = win.tile([128, NDT, FEAT], dt_qk, tag="wq")
            wk3 = win.tile([128, NDT, FEAT], dt_qk, tag="wk")
            wv3 = win.tile([128, NDT, FEAT], dt_v, tag="wv")
            wo3 = win.tile([128, 2, D], dt_o, tag="wo")
            bq3 = win.tile([128, 2, 1], fp32, tag="bq")
            bk3 = win.tile([128, 2, 1], fp32, tag="bk")
            xq3 = xin.tile([128, NDT, S], dt_qk, tag="xq")
            xk3 = xin.tile([128, NDT, S], dt_qk, tag="xk")
            xv3 = xin.tile([128, NDT, S], dt_v, tag="xv")
            nc.sync.dma_start(wk3[:], wkT.rearrange("(t p) f -> p t f", p=128))
            nc.sync.dma_start(wq3[:], wqT.rearrange("(t p) f -> p t f", p=128))
            nc.sync.dma_start(bq3[:], bq2.rearrange("(t p) o -> p t o", p=128))
            nc.sync.dma_start(bk3[:], bk2.rearrange("(t p) o -> p t o", p=128))
            for t in range(NDT):
                nc.sync.dma_start(xk3[:, t, :], xk_r[:, t, :])
                nc.sync.dma_start(xq3[:, t, :], xq_r[:, t, :])
            nc.sync.dma_start(wv3[:], wvT.rearrange("(t p) f -> p t f", p=128))
            for t in range(NDT):
                nc.sync.dma_start(xv3[:, t, :], xv_r[:, t, :])
            nc.sync.dma_start(wo3[:], woT.rearrange("(t p) j -> p t j", p=128))

            # ---- persistent intermediates ----
            qh3 = proj.tile([128, 2, S], dt_qk, tag="qh")   # pair-packed
            kh3 = proj.tile([128, 2, S], dt_qk, tag="kh")
            vha = proj.tile([128, NKT, HL, DK + 1], dt_pv, tag="vha")
            ot3 = proj.tile([128, 2, S], dt_o, tag="outT")

            nc.gpsimd.memset(vha[:, :, :, DK], 1.0)  # ones column
            # exp bias -2: headroom under fp16 max (cancels in division)
            ebias = win.tile([128, 1], fp32, tag="ebias")
            nc.gpsimd.memset(ebias[:], -2.0)

            # ---- projections ----
            def qk_proj(m):
                for x3, w3, b3, dst in ((xk3, wk3, bk3, kh3),
                                        (xq3, wq3, bq3, qh3)):
                    for n in range(NQT):
                        ps = pp.tile([128, 512], fp32, tag="acc")
                        for kt in range(NDT):
                            nc.tensor.matmul(
                                ps[:],
                                w3[:, kt, m * 128:(m + 1) * 128],
                                x3[:, kt, n * 512:(n + 1) * 512],
                                start=(kt == 0), stop=(kt == NDT - 1))
                        nc.vector.tensor_scalar_add(
                            dst[:, m, n * 512:(n + 1) * 512], ps[:], b3[:, m, :])

            def v_unit(st):
                ps = pp.tile([128, 256], fp32, tag="acc")
                for kt in range(NDT):
                    nc.tensor.matmul(
                        ps[:], xv3[:, kt, st * 128:(st + 1) * 128],
                        wv3[:, kt, :],
                        start=(kt == 0), stop=(kt == NDT - 1))
                nc.vector.tensor_copy(vha[:, st, :, 0:DK], ps[:])

            # ---- attention units ----
            e2_store = {}   # (qt, hp) -> list of e2 tiles (len NKT)
            pv_store = {}   # (qt, hp) -> (pva, pvb)

            def scores_unit(qt, hp, kt):
                s2 = ps2.tile([128, 1024], fp32, tag="s2")
                nc.tensor.matmul(
                    s2[:, 0:512],
                    kh3[0:64, hp, kt * 128:(kt + 1) * 128],
                    qh3[0:64, hp, qt * 512:(qt + 1) * 512],
                    start=True, stop=True)
                nc.tensor.matmul(
                    s2[:, 512:1024],
                    kh3[64:128, hp, kt * 128:(kt + 1) * 128],
                    qh3[64:128, hp, qt * 512:(qt + 1) * 512],
                    start=True, stop=True)
                e2 = pexp.tile([128, 1024], dt_pv, tag="e2")
                if DT_PV == "bf16":   # bf16 range: no overflow risk
                    nc.scalar.activation(
                        e2[:], s2[:],
                        mybir.ActivationFunctionType.Exp, scale=0.125)
                else:
                    nc.scalar.activation(
                        e2[:], s2[:],
                        mybir.ActivationFunctionType.Exp,
                        scale=0.125, bias=ebias[:])
                e2_store[(qt, hp)].append(e2)

            def pv_begin(qt, hp):
                pva = pspv.tile([DK + 1, 512], fp32, tag="pv")
                pvb = pspv.tile([DK + 1, 512], fp32, tag="pv")
                pv_store[(qt, hp)] = (pva, pvb)

            def pv_unit(qt, hp, kt):
                pva, pvb = pv_store[(qt, hp)]
                e2 = e2_store[(qt, hp)][kt]
                nc.tensor.matmul(
                    pva[:], vha[:, kt, 2 * hp, :], e2[:, 0:512],
                    start=(kt == 0), stop=(kt == NKT - 1))
                nc.tensor.matmul(
                    pvb[:], vha[:, kt, 2 * hp + 1, :], e2[:, 512:1024],
                    start=(kt == 0), stop=(kt == NKT - 1))

            def pv_end(qt, hp):
                pva, pvb = pv_store[(qt, hp)]
                for pv, half in ((pva, 0), (pvb, 1)):
                    # custom DVE ops must read SBUF, not PSUM
                    srow = pnrm.tile([1, 512], fp32, tag="srow")
                    nc.vector.tensor_copy(srow[:], pv[DK:DK + 1, :])
                    inv = pnrm.tile([1, 512], fp32, tag="inv")
                    nc.vector.reciprocal_approx_fast(inv[:], srow[:])
                    invb = pnrm.tile([64, 512], fp32, tag="invb")
                    nc.gpsimd.partition_broadcast(invb[:], inv[:])
                    nc.vector.tensor_tensor(
                        ot3[half * 64:(half + 1) * 64, hp,
                            qt * 512:(qt + 1) * 512],
                        pv[0:DK, :], invb[:], mybir.AluOpType.mult)

            def oproj_unit(qt, jt):
                ps = pp.tile([128, 512], fp32, tag="acc")
                for m in range(2):
                    nc.tensor.matmul(
                        ps[:], wo3[:, m, jt * 128:(jt + 1) * 128],
                        ot3[:, m, qt * 512:(qt + 1) * 512],
                        start=(m == 0), stop=(m == 1))
                po = pout.tile([128, 512], dt_o, tag="po")
                nc.vector.tensor_copy(po[:], ps[:])
                nc.sync.dma_start(
                    out_d[jt * 128:(jt + 1) * 128,
                          qt * 512:(qt + 1) * 512], po[:])

            # ---- schedule ----
            blocks = [(qt, hp) for qt in range(NQT) for hp in range(2)]
            for b in blocks:
                e2_store[b] = []

            qk_proj(0)
            for kt in range(NKT):            # e00: first ACT runway
                scores_unit(0, 0, kt)
            qk_proj(1)                       # ~14us PE under e00's ~16us ACT
            for kt in range(E01_PRE):        # extra ACT backlog before the
                scores_unit(0, 1, kt)        # vproj-heavy first block

            for i, cur in enumerate(blocks):
                qt, hp = cur
                nxt = blocks[i + 1] if i + 1 < len(blocks) else None
                # scores units of nxt still to emit, spread over 16 kt slots
                todo = []
                if nxt is not None:
                    todo = list(range(len(e2_store[nxt]), NKT))
                opq = qt - 1 if (hp == 0 and qt > 0) else None  # oproj(qt-1)
                if i == 0:
                    v_unit(0)
                pv_begin(qt, hp)
                n_todo = len(todo)
                emitted_here = 0
                for kt in range(NKT):
                    if i == 0 and kt + 1 < NKT:
                        v_unit(kt + 1)
                    pv_unit(qt, hp, kt)
                    # emit nxt's remaining scores units, paced linearly
                    # across this block's 16 kt slots
                    if todo and emitted_here < (kt + 1) * n_todo // NKT:
                        scores_unit(nxt[0], nxt[1], todo.pop(0))
                        emitted_here += 1
                    if opq is not None and kt % 2 == 1:
                        oproj_unit(opq, kt // 2)
                while todo:
                    scores_unit(nxt[0], nxt[1], todo.pop(0))
                pv_end(qt, hp)
            for jt in range(NDT):
                oproj_unit(NQT - 1, jt)

    nc.compile()
    return nc


def kernel(q, k, v, Wq, bq, Wk, bk, Wv, bv, Wo, bo, _trace=False):
    from concourse import bass_utils

    if "nc" not in _cache:
        _cache["nc"] = _build()
    nc = _cache["nc"]

    q = np.asarray(q, np.float32)
    k = np.asarray(k, np.float32)
    v = np.asarray(v, np.float32)
    Wq = np.asarray(Wq, np.float32)
    Wk = np.asarray(Wk, np.float32)
    Wv = np.asarray(Wv, np.float32)
    Wo = np.asarray(Wo, np.float32)
    bq = np.asarray(bq, np.float32)
    bk = np.asarray(bk, np.float32)
    bv = np.asarray(bv, np.float32)
    bo = np.asarray(bo, np.float32)

    d_qk, d_v, d_o = _np_dt(DT_QK), _np_dt(DT_V), _np_dt(DT_O)
    xT = {}
    for b in range(B):
        xT[("q", b)] = np.ascontiguousarray(q[b].T).astype(d_qk)
        xT[("k", b)] = np.ascontiguousarray(k[b].T).astype(d_qk)
        xT[("v", b)] = np.ascontiguousarray(v[b].T).astype(d_v)
    wT = {}
    for g in range(HG):
        sl = slice(g * FEAT, (g + 1) * FEAT)
        wT[("q", g)] = np.ascontiguousarray(Wq[sl, :].T).astype(d_qk)
        wT[("k", g)] = np.ascontiguousarray(Wk[sl, :].T).astype(d_qk)
        wT[("v", g)] = np.ascontiguousarray(Wv[sl, :].T).astype(d_v)
        wT[("o", g)] = np.ascontiguousarray(Wo[:, sl].T).astype(d_o)

    in_maps = []
    for c in range(N_CORES):
        b, g = divmod(c, HG)
        sl = slice(g * FEAT, (g + 1) * FEAT)
        in_maps.append({
            "xqT": xT[("q", b)], "xkT": xT[("k", b)], "xvT": xT[("v", b)],
            "wqT": wT[("q", g)], "wkT": wT[("k", g)], "wvT": wT[("v", g)],
            "woT": wT[("o", g)],
            "bq2": np.ascontiguousarray(bq[sl]).reshape(FEAT, 1),
            "bk2": np.ascontiguousarray(bk[sl]).reshape(FEAT, 1),
        })

    kwargs = {}
    if _trace:
        _install_profile_shim()
        kwargs = dict(trace=True, trace_cores=list(range(N_CORES)))
    res = bass_utils.run_bass_kernel_spmd(
        nc, in_maps, core_ids=list(range(N_CORES)), **kwargs)
    _cache["last_results"] = res

    final_bias = (Wo @ bv + bo).astype(np.float32)  # attn rows sum to 1
    out = np.empty((B, S, D), np.float32)
    for b in range(B):
        acc = res.results[b * HG]["partialT"].astype(np.float32)
        for g in range(1, HG):
            acc += res.results[b * HG + g]["partialT"].astype(np.float32)
        out[b] = acc.T + final_bias
    return out


def _install_profile_shim():
    """Provide antenv.axon_hooks so trace=True works under axon."""
    import sys
    import types

    import antenv

    if "antenv.axon_hooks" in sys.modules:
        return
    mod = types.ModuleType("antenv.axon_hooks")
    mod._hook = None
    mod.set_axon_ntff_profile_hook = lambda h: setattr(mod, "_hook", h)
    mod.get_axon_ntff_profile_hook = lambda: mod._hook
    sys.modules["antenv.axon_hooks"] = mod
    antenv.axon_hooks = mod
    try:
        from trn_agent_boot.trn_boot import _ntff_profile_via_ctypes
        mod.set_axon_ntff_profile_hook(
            _ntff_profile_via_ctypes("/opt/axon/libaxon_pjrt.so"))
    except Exception:
        pass


# revision 20
# speedup vs baseline: 1.1275x; 1.1002x over previous
"""MultiHeadAttention Trainium2 kernel (8 NeuronCores, Bass/Tile).

Problem: B=2, S=2048, D=1024, H=16, DK=64 fp32 MHA (torch-Linear style
projections, softmax attention, output projection).

Sharding: core c = (batch b = c//4, head-group g = c%4); each core handles
4 heads of one batch, entirely in a transposed layout (features on
partitions, sequence on the free axis):
  qhT/khT  = (W_g x^T + b)       [2 pairs x 128, 2048]
  vh       = x_v Wv_g^T          [2048, 4x65] (ones col -> row sums)
  scoresT  = khT^T qhT           per (pair, ktile, qtile) -> PSUM
  expT     = exp(scoresT/8 - 2)  ACT (bias -2 for fp16 headroom)
  rawT     = vh_aug^T expT       PV matmul; row 64 = softmax denominator
  outT     = rawT[0:64] * (1/rawT[64])
  partialT = woT^T outT          [1024, 2048] fp16 -> DRAM
Host: out[b] = sum_g partialT(b,g)^T + (Wo bv + bo).

Schedule: the kernel is ACT(exp)-throughput-bound (128 x ~1us activates).
Emission order keeps ACT saturated: qk_proj(pair0) -> scores(0,0) ->
qk_proj(pair1) -> pre-emit part of scores(0,1) -> steady blocks where
pv(cur) is kt-interleaved with scores(next) and v_proj / o_proj units
ride in the PE slack. The two K=64 score matmuls per kt auto-row-tile
(base partitions 0/64) and run concurrently on the PE.
"""

import numpy as np

B, S, D, H = 2, 2048, 1024, 16
DK = D // H          # 64
N_CORES = 8
HG = H // 4          # 4 head-groups
HL = 4               # heads per core
FEAT = HL * DK       # 256 per-core features
NQT = S // 512       # 4 query tiles
NKT = S // 128       # 16 key tiles
NDT = D // 128       # 8 contraction tiles (d-model)

# per-stage matmul operand dtypes ("fp16" | "bf16")
DT_QK = "fp16"   # x_q/x_k, Wq/Wk, qhT/khT (score operands)
DT_V = "fp16"    # x_v, Wv
DT_PV = "bf16"   # vh_aug, expT
DT_O = "fp16"    # Wo, outT (feeds final output directly)

E2_BUFS = 22     # e2 tile ring (SBUF-limited; ACT backlog cap = bufs-16)
E01_PRE = 6      # scores(0,1) units emitted right after qh pair-1

_cache = {}


def _np_dt(name):
    if name == "fp16":
        return np.float16
    import ml_dtypes
    return ml_dtypes.bfloat16


def _build():
    import concourse.mybir as mybir
    import concourse.tile as tile
    from concourse import bacc

    fp32 = mybir.dt.float32
    dt_qk = getattr(mybir.dt, "float16" if DT_QK == "fp16" else "bfloat16")
    dt_v = getattr(mybir.dt, "float16" if DT_V == "fp16" else "bfloat16")
    dt_pv = getattr(mybir.dt, "float16" if DT_PV == "fp16" else "bfloat16")
    dt_o = getattr(mybir.dt, "float16" if DT_O == "fp16" else "bfloat16")

    nc = bacc.Bacc("TRN2", target_bir_lowering=False, debug=False,
                   num_devices=N_CORES)

    xqT = nc.dram_tensor("xqT", [D, S], dt_qk, kind="ExternalInput").ap()
    xkT = nc.dram_tensor("xkT", [D, S], dt_qk, kind="ExternalInput").ap()
    xvT = nc.dram_tensor("xvT", [D, S], dt_v, kind="ExternalInput").ap()
    wqT = nc.dram_tensor("wqT", [D, FEAT], dt_qk, kind="ExternalInput").ap()
    wkT = nc.dram_tensor("wkT", [D, FEAT], dt_qk, kind="ExternalInput").ap()
    wvT = nc.dram_tensor("wvT", [D, FEAT], dt_v, kind="ExternalInput").ap()
    woT = nc.dram_tensor("woT", [FEAT, D], dt_o, kind="ExternalInput").ap()
    bq2 = nc.dram_tensor("bq2", [FEAT, 1], fp32, kind="ExternalInput").ap()
    bk2 = nc.dram_tensor("bk2", [FEAT, 1], fp32, kind="ExternalInput").ap()
    out_d = nc.dram_tensor("partialT", [D, S], dt_o, kind="ExternalOutput").ap()

    xq_r = xqT.rearrange("(t p) s -> p t s", p=128)
    xk_r = xkT.rearrange("(t p) s -> p t s", p=128)
    xv_r = xvT.rearrange("(t p) s -> p t s", p=128)

    with tile.TileContext(nc) as tc:
        with (
            tc.tile_pool(name="xin", bufs=1) as xin,
            tc.tile_pool(name="win", bufs=1) as win,
            tc.tile_pool(name="proj", bufs=1) as proj,
            tc.tile_pool(name="pexp", bufs=E2_BUFS) as pexp,
            tc.tile_pool(name="pout", bufs=4) as pout,
            tc.tile_pool(name="pnrm", bufs=1) as pnrm,
            tc.tile_pool(name="pp", bufs=2, space="PSUM") as pp,
            tc.tile_pool(name="ps2", bufs=2, space="PSUM") as ps2,
            tc.tile_pool(name="pspv", bufs=2, space="PSUM") as pspv,
        ):
            # ---- load inputs. Order = consumption order: q/k weights,
            # then xk/xq interleaved (first exp gates on these), then the
            # v path, then wo (only needed ~60us in).
            wq3 = win.tile([128, NDT, FEAT], dt_qk, tag="wq")
            wk3 = win.tile([128, NDT, FEAT], dt_qk, tag="wk")
            wv3 = win.tile([128, NDT, FEAT], dt_v, tag="wv")
            wo3 = win.tile([128, 2, D], dt_o, tag="wo")
            bq3 = win.tile([128, 2, 1], fp32, tag="bq")
            bk3 = win.tile([128, 2, 1], fp32, tag="bk")
            xq3 = xin.tile([128, NDT, S], dt_qk, tag="xq")
            xk3 = xin.tile([128, NDT, S], dt_qk, tag="xk")
            xv3 = xin.tile([128, NDT, S], dt_v, tag="xv")
            # arrival order tracks first use: kh units need xk+wk only,
            # qh + first scores need xq+wq, then the v path, then wo
            nc.sync.dma_start(xk3[:], xk_r)
            nc.sync.dma_start(wk3[:], wkT.rearrange("(t p) f -> p t f", p=128))
            nc.sync.dma_start(bk3[:], bk2.rearrange("(t p) o -> p t o", p=128))
            nc.sync.dma_start(xq3[:], xq_r)
            nc.sync.dma_start(wq3[:], wqT.rearrange("(t p) f -> p t f", p=128))
            nc.sync.dma_start(bq3[:], bq2.rearrange("(t p) o -> p t o", p=128))
            nc.sync.dma_start(wv3[:], wvT.rearrange("(t p) f -> p t f", p=128))
            nc.sync.dma_start(xv3[:], xv_r)
            nc.sync.dma_start(wo3[:], woT.rearrange("(t p) j -> p t j", p=128))

            # ---- persistent intermediates ----
            qh3 = proj.tile([128, 2, S], dt_qk, tag="qh")   # pair-packed
            kh3 = proj.tile([128, 2, S], dt_qk, tag="kh")
            vha = proj.tile([128, NKT, HL, DK + 1], dt_pv, tag="vha")
            ot3 = proj.tile([128, 2, S], dt_o, tag="outT")

            nc.gpsimd.memset(vha[:, :, :, DK], 1.0)  # ones column
            # exp bias -2: headroom under fp16 max (cancels in division)
            ebias = win.tile([128, 1], fp32, tag="ebias")
            nc.gpsimd.memset(ebias[:], -2.0)

            # ---- projections ----
            def proj_n_unit(m, n, x3, w3, b3, dst):
                ps = pp.tile([128, 512], fp32, tag="acc")
                for kt in range(NDT):
                    nc.tensor.matmul(
                        ps[:],
                        w3[:, kt, m * 128:(m + 1) * 128],
                        x3[:, kt, n * 512:(n + 1) * 512],
                        start=(kt == 0), stop=(kt == NDT - 1))
                nc.vector.tensor_scalar_add(
                    dst[:, m, n * 512:(n + 1) * 512], ps[:], b3[:, m, :])

            def qk_proj(m):
                for x3, w3, b3, dst in ((xk3, wk3, bk3, kh3),
                                        (xq3, wq3, bq3, qh3)):
                    for n in range(NQT):
                        proj_n_unit(m, n, x3, w3, b3, dst)

            def v_unit(st):
                ps = pp.tile([128, 256], fp32, tag="acc")
                for kt in range(NDT):
                    nc.tensor.matmul(
                        ps[:], xv3[:, kt, st * 128:(st + 1) * 128],
                        wv3[:, kt, :],
                        start=(kt == 0), stop=(kt == NDT - 1))
                nc.vector.tensor_copy(vha[:, st, :, 0:DK], ps[:])

            # ---- attention units ----
            e2_store = {}   # (qt, hp) -> list of e2 tiles (len NKT)
            pv_store = {}   # (qt, hp) -> (pva, pvb)

            def scores_unit(qt, hp, kt):
                s2 = ps2.tile([128, 1024], fp32, tag="s2")
                nc.tensor.matmul(
                    s2[:, 0:512],
                    kh3[0:64, hp, kt * 128:(kt + 1) * 128],
                    qh3[0:64, hp, qt * 512:(qt + 1) * 512],
                    start=True, stop=True)
                nc.tensor.matmul(
                    s2[:, 512:1024],
                    kh3[64:128, hp, kt * 128:(kt + 1) * 128],
                    qh3[64:128, hp, qt * 512:(qt + 1) * 512],
                    start=True, stop=True)
                e2 = pexp.tile([128, 1024], dt_pv, tag="e2")
                if DT_PV == "bf16":   # bf16 range: no overflow risk
                    nc.scalar.activation(
                        e2[:], s2[:],
                        mybir.ActivationFunctionType.Exp, scale=0.125)
                else:
                    nc.scalar.activation(
                        e2[:], s2[:],
                        mybir.ActivationFunctionType.Exp,
                        scale=0.125, bias=ebias[:])
                e2_store[(qt, hp)].append(e2)

            def pv_begin(qt, hp, pool=None):
                if pool is None:
                    pva = pspv.tile([DK + 1, 512], fp32, tag="pv")
                    pvb = pspv.tile([DK + 1, 512], fp32, tag="pv")
                else:  # borrow [128,512] accs (e.g. pp) and use 65 rows
                    pva_f = pool.tile([128, 512], fp32, tag="acc")
                    pvb_f = pool.tile([128, 512], fp32, tag="acc")
                    pva = pva_f[0:DK + 1, :]
                    pvb = pvb_f[0:DK + 1, :]
                pv_store[(qt, hp)] = (pva, pvb)

            def pv_unit(qt, hp, kt):
                pva, pvb = pv_store[(qt, hp)]
                e2 = e2_store[(qt, hp)][kt]
                nc.tensor.matmul(
                    pva[:], vha[:, kt, 2 * hp, :], e2[:, 0:512],
                    start=(kt == 0), stop=(kt == NKT - 1))
                nc.tensor.matmul(
                    pvb[:], vha[:, kt, 2 * hp + 1, :], e2[:, 512:1024],
                    start=(kt == 0), stop=(kt == NKT - 1))

            def pv_end(qt, hp):
                pva, pvb = pv_store[(qt, hp)]
                # stage-major order so the two halves' chains overlap
                # (copy-b runs while recip-a, gpsimd bcast overlaps DVE)
                srow, inv, invb = [], [], []
                for half, pv in enumerate((pva, pvb)):
                    # custom DVE ops must read SBUF, not PSUM
                    s = pnrm.tile([1, 512], fp32, tag="srow")
                    nc.vector.tensor_copy(s[:], pv[DK:DK + 1, :])
                    srow.append(s)
                for half in range(2):
                    iv = pnrm.tile([1, 512], fp32, tag="inv")
                    nc.vector.reciprocal_approx_fast(iv[:], srow[half][:])
                    inv.append(iv)
                for half in range(2):
                    ib = pnrm.tile([64, 512], fp32, tag="invb")
                    nc.gpsimd.partition_broadcast(ib[:], inv[half][:])
                    invb.append(ib)
                for half, pv in enumerate((pva, pvb)):
                    nc.vector.tensor_tensor(
                        ot3[half * 64:(half + 1) * 64, hp,
                            qt * 512:(qt + 1) * 512],
                        pv[0:DK, :], invb[half][:], mybir.AluOpType.mult)

            def oproj_unit(qt, jt, acc=None):
                if acc is None:
                    acc_t = pp.tile([128, 512], fp32, tag="acc")
                    acc = acc_t[:]
                ps = acc
                for m in range(2):
                    nc.tensor.matmul(
                        ps, wo3[:, m, jt * 128:(jt + 1) * 128],
                        ot3[:, m, qt * 512:(qt + 1) * 512],
                        start=(m == 0), stop=(m == 1))
                po = pout.tile([128, 512], dt_o, tag="po")
                nc.vector.tensor_copy(po[:], ps)
                nc.sync.dma_start(
                    out_d[jt * 128:(jt + 1) * 128,
                          qt * 512:(qt + 1) * 512], po[:])

            # ---- schedule ----
            blocks = [(qt, hp) for qt in range(NQT) for hp in range(2)]
            for b in blocks:
                e2_store[b] = []

            # Front: both kh pairs run while only xk is resident; qh pair 0
            # + scores(0,0) as xq lands; qh pair 1 fits under e00's ~17us
            # of queued ACT work, so ACT never stalls after its first exp.
            for m in range(2):
                for n in range(NQT):
                    proj_n_unit(m, n, xk3, wk3, bk3, kh3)
            for n in range(NQT):
                proj_n_unit(0, n, xq3, wq3, bq3, qh3)
                for kt in range(4 * n, 4 * n + 4):
                    scores_unit(0, 0, kt)
            for n in range(NQT):
                proj_n_unit(1, n, xq3, wq3, bq3, qh3)
            for kt in range(E01_PRE):        # extra ACT backlog before the
                scores_unit(0, 1, kt)        # vproj-heavy first block

            NB = len(blocks)
            for i, cur in enumerate(blocks):
                qt, hp = cur
                last = (i == NB - 1)
                nxt = blocks[i + 1] if not last else None
                # scores units of nxt still to emit, paced over 16 kt slots
                todo = list(range(len(e2_store[nxt]), NKT)) if nxt else []
                n_todo = len(todo)
                emitted_here = 0
                if i == 0:
                    v_unit(0)
                if not last:
                    pv_begin(qt, hp)
                for kt in range(NKT):
                    if i == 0 and kt + 1 < NKT:
                        v_unit(kt + 1)
                    if last:
                        # second half of pv(3,1); first half ran in the
                        # previous block at ACT cadence (pp-pool accs)
                        if kt < 8:
                            pv_unit(qt, hp, 8 + kt)
                    else:
                        pv_unit(qt, hp, kt)
                    if todo and emitted_here < (kt + 1) * n_todo // NKT:
                        scores_unit(nxt[0], nxt[1], todo.pop(0))
                        emitted_here += 1
                    if i in (2, 4) and kt % 2 == 1:
                        oproj_unit(qt - 1, kt // 2)        # oproj(0)/(1)
                    if i == NB - 2:
                        if kt < 8:
                            oproj_unit(2, kt)              # oproj(2) early
                        else:
                            if kt == 8:
                                pv_begin(*blocks[i + 1], pool=pp)
                            pv_unit(blocks[i + 1][0], blocks[i + 1][1],
                                    kt - 8)
                while todo:
                    scores_unit(nxt[0], nxt[1], todo.pop(0))
                pv_end(qt, hp)
            # oproj(3): pipeline across 4 psum accs (pp ring + idle ps2)
            for jt in range(NDT):
                if jt % 2 == 0:
                    acc_s2 = ps2.tile([128, 1024], fp32, tag="s2")
                    acc = acc_s2[:, 0:512]
                else:
                    acc = None
                oproj_unit(NQT - 1, jt, acc)

    nc.compile()
    return nc


def kernel(q, k, v, Wq, bq, Wk, bk, Wv, bv, Wo, bo, _trace=False):
    from concourse import bass_utils

    if "nc" not in _cache:
        _cache["nc"] = _build()
    nc = _cache["nc"]

    q = np.asarray(q, np.float32)
    k = np.asarray(k, np.float32)
    v = np.asarray(v, np.float32)
    Wq = np.asarray(Wq, np.float32)
    Wk = np.asarray(Wk, np.float32)
    Wv = np.asarray(Wv, np.float32)
    Wo = np.asarray(Wo, np.float32)
    bq = np.asarray(bq, np.float32)
    bk = np.asarray(bk, np.float32)
    bv = np.asarray(bv, np.float32)
    bo = np.asarray(bo, np.float32)

    d_qk, d_v, d_o = _np_dt(DT_QK), _np_dt(DT_V), _np_dt(DT_O)
    xT = {}
    for b in range(B):
        xT[("q", b)] = np.ascontiguousarray(q[b].T).astype(d_qk)
        xT[("k", b)] = np.ascontiguousarray(k[b].T).astype(d_qk)
        xT[("v", b)] = np.ascontiguousarray(v[b].T).astype(d_v)
    wT = {}
    for g in range(HG):
        sl = slice(g * FEAT, (g + 1) * FEAT)
        wT[("q", g)] = np.ascontiguousarray(Wq[sl, :].T).astype(d_qk)
        wT[("k", g)] = np.ascontiguousarray(Wk[sl, :].T).astype(d_qk)
        wT[("v", g)] = np.ascontiguousarray(Wv[sl, :].T).astype(d_v)
        wT[("o", g)] = np.ascontiguousarray(Wo[:, sl].T).astype(d_o)

    in_maps = []
    for c in range(N_CORES):
        b, g = divmod(c, HG)
        sl = slice(g * FEAT, (g + 1) * FEAT)
        in_maps.append({
            "xqT": xT[("q", b)], "xkT": xT[("k", b)], "xvT": xT[("v", b)],
            "wqT": wT[("q", g)], "wkT": wT[("k", g)], "wvT": wT[("v", g)],
            "woT": wT[("o", g)],
            "bq2": np.ascontiguousarray(bq[sl]).reshape(FEAT, 1),
            "bk2": np.ascontiguousarray(bk[sl]).reshape(FEAT, 1),
        })

    kwargs = {}
    if _trace:
        _install_profile_shim()
        kwargs = dict(trace=True, trace_cores=list(range(N_CORES)))
    res = bass_utils.run_bass_kernel_spmd(
        nc, in_maps, core_ids=list(range(N_CORES)), **kwargs)
    _cache["last_results"] = res

    final_bias = (Wo @ bv + bo).astype(np.float32)  # attn rows sum to 1
    out = np.empty((B, S, D), np.float32)
    for b in range(B):
        acc = res.results[b * HG]["partialT"].astype(np.float32)
        for g in range(1, HG):
            acc += res.results[b * HG + g]["partialT"].astype(np.float32)
        out[b] = acc.T + final_bias
    return out


def _install_profile_shim():
    """Provide antenv.axon_hooks so trace=True works under axon."""
    import sys
    import types

    import antenv

    if "antenv.axon_hooks" in sys.modules:
        return
    mod = types.ModuleType("antenv.axon_hooks")
    mod._hook = None
    mod.set_axon_ntff_profile_hook = lambda h: setattr(mod, "_hook", h)
    mod.get_axon_ntff_profile_hook = lambda: mod._hook
    sys.modules["antenv.axon_hooks"] = mod
    antenv.axon_hooks = mod
    try:
        from trn_agent_boot.trn_boot import _ntff_profile_via_ctypes
        mod.set_axon_ntff_profile_hook(
            _ntff_profile_via_ctypes("/opt/axon/libaxon_pjrt.so"))
    except Exception:
        pass
